# revision 67
# baseline (speedup 1.0000x reference)
# Multi-head attention (B=2, S=2048, D=1024, H=16, head_dim=64) with bool mask,
# sharded across 8 TRN2 NeuronCores: core c -> batch c//4, heads 4*(c%4)..4*(c%4)+3.
#
# Per-core device kernel (scores computed transposed: scoresT[k, q]):
#   scoresT = K @ Q^T              (PE bf16, per 128-k strip, into PSUM)
#   at      = exp(scoresT/8)*nmT   split between two engines:
#              - ACT strips: ACT exp (scale=1/8) -> fp16, then DVE/Pool mask mul
#              - DVE strips: custom DVE op1 p=poly4(x)~e^(x/64) -> fp16,
#                custom DVE op2 at=(p^2)^2)^2*mask (mask fused; m^8=m)
#   psO     = [V|1]^T @ at         (PE fp16 accumulate over strips; row 64 = Z)
#   evac    = copy psO -> SBUF bf16 (DVE), DMA out (numerator+Z, unnormalized)
# Host side: shard inputs, pre-transpose Q/K per head (bf16), build V'=[V|1]
# (fp16), inverted-transposed mask (fp16); after run: out = (num/Z)^T merge.
#
# Scheduling: engine wait-queues are head-blocking FIFOs, so every stream is
# emitted in dependency-readiness order (QK first per slot, AV chunks delayed
# 3-5 slots past their strip, budget-limited to 3 chunks/slot). Groups run
# qg-major so the q-window-1 mask DMA deadline relaxes to ~slot 64. The last
# two strips fold the mask into psS via a 10000*identity matmul + exp bias
# -1250 to shorten the tail. The custom DVE ops offload ~1/5 of the exp work
# from the ACT engine (the baseline bottleneck, ~133us busy) to the DVE ALU
# pipeline; Pool takes 1/4 of the mask multiplies. PE/ACT/DVE all land at
# ~109-112us busy. TimelineSim: 158169 ns (baseline) -> 129554 ns.

import sys

import numpy as np

for _p in ("/opt/trn_rl_repo",):
    if _p not in sys.path:
        sys.path.insert(0, _p)

import ml_dtypes

import concourse.bass as bass  # noqa: F401  (engine types reachable via nc)
import concourse.tile as tile
from concourse import bacc, mybir
from concourse.bass_utils import run_bass_kernel_spmd

F32 = mybir.dt.float32
BF16 = mybir.dt.bfloat16
FP16 = mybir.dt.float16

S = 2048          # sequence length
HD = 64           # head dim
HPC = 4           # heads per core
NCORES = 8
B = 2
H = 16
D = H * HD

# e^(x/64) ~= 1 + c1 x + c2 x^2 + c3 x^3 + c4 x^4 on |x| <= 52 (~6.5 sigma of
# the N(0,64) score distribution); rel err 2.5e-4, x8 through the squarings
# -> ~2e-3 weight noise on DVE strips (output impact ~0.1%).
POLY_C = (1.5610800e-02, 1.2226440e-04, 6.6047774e-07, 2.4435055e-09)

_DVE_OPS = {}


def _register_dve_ops():
    """Register the two custom DVE ops (idempotent). Uses the documented
    extension point (dve_ops.OPS) at runtime since the repo is read-only."""
    if _DVE_OPS:
        return _DVE_OPS
    from concourse import dve_ops
    from concourse.dve_spec import (
        C0,
        C1,
        C2,
        C3,
        One,
        Spec,
        Src0,
        Src1,
        _has_src1,
        _spill_c3_to_src1,
        lower,
    )
    from concourse.dve_uop import DveOpSpec

    def _mk(name, body, reference, subdim=False):
        existing = {op.name: op for op in dve_ops.OPS}
        if name in existing:
            _DVE_OPS[name] = existing[name]
            return existing[name]
        spec = Spec(body=body, reference=reference)
        shas = {}
        for ver in ("v3", "v4"):
            try:
                uops = lower(spec, ver=ver)
                probe = DveOpSpec(
                    name=name, opcode=31, uops=uops, rd1_en=_has_src1(spec)
                )
                shas[ver] = probe.sha(ver)
            except Exception:
                pass
        op = dve_ops.DveOp(name, spec, subdim=subdim, uops_sha=shas)
        dve_ops.OPS.append(op)
        dve_ops.CUSTOM_DVE_SPECS[name] = spec
        dve_ops._SUB_OPCODE_FOR_NAME[name] = dve_ops._CUSTOM_DVE_ROW_BASE + (
            len(dve_ops.OPS) - 1
        )
        assert dve_ops._SUB_OPCODE_FOR_NAME[name] < 0x20
        _DVE_OPS[name] = op
        return op

    # op1: p = 1 + x(c1 + x(c2 + x(c3 + x c4)))   [8 ALU stages]
    # binding: s0=c4, s1=c3, imm2=c2, in1=[P,1] c1 (spilled C3)
    body1 = ((((Src0 * C0 + C1) * Src0 + C2) * Src0 + C3) * Src0) + One
    body1 = _spill_c3_to_src1(body1)

    def ref1(in0, in1, s0, s1, imm2):
        x = np.asarray(in0, np.float32)
        c1v = np.asarray(in1, np.float32)
        return ((((x * s0 + s1) * x + imm2) * x + c1v) * x) + np.float32(1.0)

    _mk("ANT_EXP64_P4", body1, ref1)

    # op2: at = (((p^2)^2)^2) * mask   [4 ALU stages]
    body2 = (Src0 * Src0)
    body2 = body2 * body2
    body2 = body2 * body2
    body2 = body2 * Src1

    def ref2(in0, in1, s0, s1, imm2):
        p = np.asarray(in0, np.float32)
        p8 = ((p * p) ** 2) ** 2
        return p8 * np.asarray(in1, np.float32)

    _mk("ANT_SQ3_MUL", body2, ref2)
    return _DVE_OPS


# Strips (by ks index) whose exp runs on the DVE via the custom ops, per
# group; and ACT strips whose mask-mul runs on Pool instead of DVE.
DVE_KS = (2, 6, 10)             # 3 per group x 8 groups = 24 DVE strips
POOL_MASK_KS = (0, 3, 7, 11)    # mask muls moved off DVE to the idle Pool


def build_program(s=S, reps=1, dve_ks=DVE_KS, pool_mask_ks=POOL_MASK_KS,
                  dve_ks0=(2, 6, 10), extra=(), n_psS=3, n_psO=1,
                  n_warm=20, av_budget=3, act_d=3, dve_d=5, pool_d=4,
                  evac_single=True, last_d=2, apool_bufs=20,
                  pool_mask_early=None, n_early=2, fill_pad=1,
                  last_pool_pad=1, last_dve_pull=0, bnd_budget=2,
                  tail_par_evac=False, bnd_slots=(15, 0)):
    # extra: iterable of (group, ks) additionally offloaded to the DVE
    """Build the single-core SPMD program. Returns the compiled Bacc object."""
    ops = _register_dve_ops()
    op_poly = ops["ANT_EXP64_P4"]
    op_sq3 = ops["ANT_SQ3_MUL"]

    nc = bacc.Bacc()

    KS = s // 128            # number of k strips
    QG = 1024 if s >= 1024 else s   # q group width
    NQG = s // QG            # q groups
    NQC = max(QG // 512, 1)  # 512-wide matmul chunks per q group
    QC = min(512, QG)        # matmul chunk width
    NG = HPC * NQG           # groups per core

    c4, c3, c2, c1 = POLY_C[3], POLY_C[2], POLY_C[1], POLY_C[0]

    qkT_d = nc.declare_dram_parameter("qkT", [2, HPC * HD, s], BF16, isOutput=False)
    vp_d = nc.declare_dram_parameter("vp", [128, KS, HPC, HD + 1], FP16, isOutput=False)
    nmT_d = nc.declare_dram_parameter("nmT", [s, s], FP16, isOutput=False)
    out_d = nc.declare_dram_parameter("out", [NG, HD + 1, QG], BF16, isOutput=True)

    nm_view = nmT_d[:].rearrange("(ks p) q -> p ks q", p=128)

    with tile.TileContext(nc) as tc:
        with (
            tc.tile_pool(name="const", bufs=1) as const,
            tc.tile_pool(name="wq", bufs=1) as wq,
            tc.tile_pool(name="attn", bufs=apool_bufs) as apool,
            tc.tile_pool(name="ppool", bufs=3) as ppool,
            tc.tile_pool(name="evac", bufs=2) as epool,
            tc.tile_pool(name="psS", bufs=n_psS, space="PSUM") as psS_pool,
            tc.tile_pool(name="psO", bufs=n_psO, space="PSUM") as psO_pool,
        ):
            # [P,1] scalar for the spilled 4th poly coefficient.
            c1t = const.tile([128, 1], F32)
            nc.vector.memset(c1t, c1)
            # [P,1] exp bias for the mask-folded tail strips
            nbias = const.tile([128, 1], F32)
            nc.vector.memset(nbias, -1250.0)

            # Preload the exp table while the first DMAs stream.
            warm = const.tile([128, 1], F32)
            nc.vector.memset(warm, 0.0)
            nc.scalar.activation(warm, warm, mybir.ActivationFunctionType.Exp)

            # Warm the PE HAM clock gate: ~3us of dummy matmuls so the first
            # real QKs run at full frequency.
            zb = const.tile([128, 128], BF16)
            nc.vector.memset(zb, 0.0)
            for _ in range(n_warm):
                wmm = psS_pool.tile([128, QG], F32, tag="psS")
                nc.tensor.matmul(
                    wmm[:, :128], lhsT=zb[0:64, :], rhs=zb[0:64, :],
                    start=True, stop=True,
                )

            # 10000 * identity (fp16): folds the mask into psS for the tail
            # strips via a PE copy-matmul, so their at needs no mask multiply
            from concourse.masks import make_identity

            identK = const.tile([128, 128], FP16)
            make_identity(nc, identK)
            nc.gpsimd.tensor_scalar_mul(identK, identK, 10000.0)

            def emit_body():
                # Input staging tiles
                qks = []
                for pair in range(HPC // 2):
                    qk = wq.tile([128, 2, s], BF16, tag=f"qkT{pair}")
                    qks.append(qk)
                vp_sb = wq.tile([128, KS, HPC, HD + 1], FP16, tag="vp")
                nm_sb = wq.tile([128, KS * s], FP16, tag="nm")

                def qk_src(pair):
                    return qkT_d[:, 128 * pair:128 * pair + 128, :].rearrange(
                        "t p s -> p t s"
                    )

                # DMA schedule: sync (SP HWDGE) carries the critical-path
                # early inputs; pool (SWDGE) carries late inputs + outputs.
                KH = KS // 2
                # k half of pair0 (cols 0:s) and q half cols of group 0.
                # Priority order for the serialized DMA-engine resource:
                # first QK operands for strips 0-3, then mask strips as the
                # consumer stream needs them, V' halves and the later q/k
                # windows just in time, second head pair and the qg1 mask
                # halves last.
                HQ = QG // 2
                nm_dst = lambda ks, a, b: nm_sb[:, ks * s + a:ks * s + b]

                def dq(dst, src):
                    nc.sync.dma_start(out=dst, in_=src)

                # just-in-time order for the serialized DMA-engine resource:
                # pair0 k/q + nm-a first (slots 0-31), pair1 next (slot 32+),
                # the qg1 q-columns and mask halves last (slot 64+)
                dq(qks[0][:, 1, 0:HQ], qk_src(0)[:, 1, 0:HQ])    # k strips 0-3
                dq(qks[0][:, 0, 0:HQ], qk_src(0)[:, 0, 0:HQ])    # q chunk 0
                dq(qks[0][:, 0, HQ:QG], qk_src(0)[:, 0, HQ:QG])  # q chunk 1
                dq(nm_dst(0, 0, QG), nm_view[:, 0, 0:QG])
                dq(qks[0][:, 1, HQ:QG], qk_src(0)[:, 1, HQ:QG])  # k strips 4-7
                dq(nm_dst(1, 0, QG), nm_view[:, 1, 0:QG])
                dq(vp_sb[:, :KH], vp_d[:, :KH])
                dq(nm_dst(2, 0, QG), nm_view[:, 2, 0:QG])
                dq(nm_dst(3, 0, QG), nm_view[:, 3, 0:QG])
                dq(nm_dst(4, 0, QG), nm_view[:, 4, 0:QG])
                if s > QG:
                    dq(qks[0][:, 1, QG:s], qk_src(0)[:, 1, QG:s])  # k 8-15
                dq(nm_dst(5, 0, QG), nm_view[:, 5, 0:QG])
                dq(nm_dst(6, 0, QG), nm_view[:, 6, 0:QG])
                dq(vp_sb[:, KH:], vp_d[:, KH:])
                for ks in range(7, KS):
                    dq(nm_dst(ks, 0, QG), nm_view[:, ks, 0:QG])
                for pair in range(1, HPC // 2):
                    dq(qks[pair][:, 1, :], qk_src(pair)[:, 1, :])
                    dq(qks[pair][:, 0, 0:QG], qk_src(pair)[:, 0, 0:QG])
                if s > QG:
                    dq(qks[0][:, 0, QG:s], qk_src(0)[:, 0, QG:s])
                    for pair in range(1, HPC // 2):
                        dq(qks[pair][:, 0, QG:s], qk_src(pair)[:, 0, QG:s])
                    for ks in range(KS):
                        dq(nm_dst(ks, QG, s), nm_view[:, ks, QG:s])

                # qg-major: all head groups sweep q-window 0 first, pushing
                # the qg1 mask-DMA deadline from ~slot 16 to ~slot 64
                groups = [(h, qg) for qg in range(NQG) for h in range(HPC)]
                NT = NG * KS  # total strips

                # per-group / scheduler state
                psO = {}            # g -> psum tile
                n_chunks = {}       # g -> AV chunks emitted
                av_q = []           # (min_slot, g, h, qg, ks, at, qc)
                dve_q = []          # (min_slot, kind, payload) for DVE stream
                AV_BUDGET = av_budget  # AV chunks per slot (> steady-state 2
                                       # so backlogs drain)

                ev_tiles = {}

                def emit_av_chunk(g, h, qg, ks, at, qc):
                    if not n_chunks.get(g):
                        psO[g] = psO_pool.tile(
                            [HD + 1, QG], F32, tag="psO", name=f"psO{g}"
                        )
                        n_chunks[g] = [0] * NQC
                    n_chunks[g][qc] += 1
                    nc.tensor.matmul(
                        psO[g][:, qc * QC:(qc + 1) * QC],
                        lhsT=vp_sb[:, ks, h, :],
                        rhs=at[:, qc * QC:(qc + 1) * QC],
                        start=n_chunks[g][qc] == 1,
                        stop=n_chunks[g][qc] == KS,
                    )
                    # column-range qc complete -> its evac can go
                    return n_chunks[g][qc] == KS

                def emit_evac(g, qc):
                    if g not in ev_tiles:
                        ev_tiles[g] = epool.tile(
                            [HD + 1, QG], BF16, tag="ev", name=f"ev{g}"
                        )
                    ev = ev_tiles[g]
                    if evac_single:
                        if qc == 0:
                            return
                        nc.vector.tensor_copy(ev, psO[g])
                        nc.sync.dma_start(out=out_d[g], in_=ev[:])
                        return
                    sl = slice(qc * QC, (qc + 1) * QC)
                    nc.vector.tensor_copy(ev[:, sl], psO[g][:, sl])
                    eng = nc.scalar if (g == NG - 1 and qc == 1) else nc.sync
                    eng.dma_start(out=out_d[g][:, sl], in_=ev[:, sl])

                for t in range(NT + 8):
                    # 1) QK strip -> psS (its psS-slot dep resolves earliest)
                    if t < NT:
                        g, ks = divmod(t, KS)
                        h, qg = groups[g]
                        q0 = qg * QG
                        base = 64 * (h % 2)
                        qt_r = qks[h // 2][:, 0, :]
                        kt_r = qks[h // 2][:, 1, :]
                        # group 0's DVE strips sit later so the pipeline-fill
                        # mask backlog on the DVE has drained by then
                        is_dve = ks in (dve_ks0 if g == 0 else dve_ks) or (
                            (g, ks) in extra
                        )
                        # tail strips: fold the mask into psS via a PE
                        # copy-matmul (10000*nm) + exp bias so the at needs
                        # no separate mask multiply after the final exp
                        trick = g == NG - 1 and ks >= KS - 2

                        psS = psS_pool.tile([128, QG], F32, tag="psS")
                        if trick:
                            for qc in range(NQC):
                                nc.tensor.matmul(
                                    psS[:, qc * QC:(qc + 1) * QC],
                                    lhsT=identK[:, :],
                                    rhs=nm_sb[:, ks * s + q0 + qc * QC:
                                              ks * s + q0 + (qc + 1) * QC],
                                    start=True,
                                    stop=False,
                                    skip_group_check=True,
                                )
                        for qc in range(NQC):
                            nc.tensor.matmul(
                                psS[:, qc * QC:(qc + 1) * QC],
                                lhsT=kt_r[base:base + HD,
                                          ks * 128:(ks + 1) * 128],
                                rhs=qt_r[base:base + HD,
                                         q0 + qc * QC:q0 + (qc + 1) * QC],
                                start=not trick,
                                stop=True,
                                skip_group_check=trick,
                            )

                    # 2) deferred DVE-stream work due now: these became ready
                    # before this slot's poly (DVE wait queues are FIFO, so
                    # emission order must match readiness order)
                    while dve_q and dve_q[0][0] <= t:
                        _, kind, pl = dve_q.pop(0)
                        if kind == "mul":
                            at_, nm_ = pl
                            nc.vector.tensor_mul(at_, at_, nm_)
                        elif kind == "sq3":
                            at_, p_, nm_ = pl
                            nc.vector._custom_dve(
                                op_sq3, out=at_[:], in0=p_[:], in1=nm_,
                            )
                        elif kind == "tevac":
                            tg, tqc = pl
                            if tg not in ev_tiles:
                                ev_tiles[tg] = epool.tile(
                                    [HD + 1, QG], BF16, tag="ev",
                                    name=f"ev{tg}"
                                )
                            evt = ev_tiles[tg]
                            tsl = slice(tqc * QC, (tqc + 1) * QC)
                            if tqc == 0:
                                nc.vector.tensor_copy(evt[:, tsl],
                                                      psO[tg][:, tsl])
                                nc.sync.dma_start(out=out_d[tg][:, tsl],
                                                  in_=evt[:, tsl])
                            else:
                                nc.scalar.copy(evt[:, tsl], psO[tg][:, tsl])
                                nc.scalar.dma_start(out=out_d[tg][:, tsl],
                                                    in_=evt[:, tsl])
                        else:  # evac
                            emit_evac(*pl)

                    if t < NT:
                        nm_ap = nm_sb[:, ks * s + q0:ks * s + q0 + QG]
                        at = apool.tile([128, QG], FP16, tag="at")
                        if is_dve:
                            p = ppool.tile([128, QG], FP16, tag="p")
                            nc.vector._custom_dve(
                                op_poly, out=p[:], in0=psS[:], in1=c1t[:],
                                s0=c4, s1=c3, imm2=c2,
                            )
                            dve_q.append((t + 2, "sq3", (at, p, nm_ap)))
                            delay = dve_d
                            if ks == (dve_ks0 if g == 0 else dve_ks)[-1]:
                                delay = dve_d - last_dve_pull
                        elif trick:
                            nc.scalar.activation(
                                at, psS, mybir.ActivationFunctionType.Exp,
                                scale=0.125, bias=nbias[:],
                            )
                            delay = 1
                        elif t < 2:
                            # pipeline fill: exp per 512-half so the first
                            # activation starts as soon as QK chunk 0 lands
                            for qc in range(NQC):
                                sl = slice(qc * QC, (qc + 1) * QC)
                                nc.scalar.activation(
                                    at[:, sl], psS[:, sl],
                                    mybir.ActivationFunctionType.Exp,
                                    scale=0.125,
                                )
                            dve_q.append((t + 1, "mul", (at, nm_ap)))
                            delay = act_d
                        else:
                            nc.scalar.activation(
                                at, psS, mybir.ActivationFunctionType.Exp,
                                scale=0.125,
                            )
                            pm = pool_mask_ks if (
                                pool_mask_early is None or g >= n_early
                            ) else pool_mask_early
                            if ks in pm:
                                nc.gpsimd.tensor_mul(at, at, nm_ap)
                                delay = pool_d
                                if ks == pm[-1]:
                                    # the group's last Pool strip: keep its
                                    # AVs behind the next group's first QK
                                    delay = pool_d + last_pool_pad
                            else:
                                dve_q.append((t + 1, "mul", (at, nm_ap)))
                                delay = act_d
                                if g < 2 and 3 <= ks <= 8:
                                    # fill phase: DVE mul stream lags; keep
                                    # these AVs behind the QKs in the FIFO
                                    delay = act_d + fill_pad
                        if g == NG - 1 and ks >= KS - 4:
                            # last group's tail: nothing behind these AVs on
                            # the PE, so drain them as early as possible
                            delay = min(delay, last_d) if not trick else 1
                        for qc in range(NQC):
                            av_q.append((t + delay, g, h, qg, ks, at, qc))
                        av_q.sort(key=lambda x: x[0])
                        dve_q.sort(key=lambda x: x[0])

                    # 3) due AV chunks, budget-limited for PE smoothness;
                    # at boundary slots the QK stream has priority
                    budget = AV_BUDGET if t < NT else 8
                    if t < NT and bnd_budget and t % KS in bnd_slots:
                        budget = bnd_budget
                    while av_q and av_q[0][0] <= t and budget > 0:
                        _, pg, ph, pqg, pks, pat, pqc = av_q.pop(0)
                        budget -= 1
                        done_qc = emit_av_chunk(pg, ph, pqg, pks, pat, pqc)
                        if done_qc:
                            if tail_par_evac and pg == NG - 1:
                                dve_q.append((t, "tevac", (pg, pqc)))
                            else:
                                dve_q.append((t + 1, "evac", (pg, pqc)))
                            dve_q.sort(key=lambda x: x[0])

            for _ in range(reps):
                emit_body()
    nc.compile()
    return nc


_CACHE = {}


def _get_nc():
    if "nc" not in _CACHE:
        _CACHE["nc"] = build_program()
    return _CACHE["nc"]


def make_in_maps(q, k, v, mask, s=S):
    """Shard full inputs into 8 per-core input maps (host-side layout prep)."""
    q = np.asarray(q, dtype=np.float32)
    k = np.asarray(k, dtype=np.float32)
    v = np.asarray(v, dtype=np.float32)
    mask = np.asarray(mask)
    nh = q.shape[-1] // HD
    ks_n = s // 128
    in_maps = []
    for c in range(NCORES):
        b, g = divmod(c, NCORES // B)
        h0 = HPC * g
        qs = q[b].reshape(s, nh, HD)[:, h0:h0 + HPC, :]      # [s, HPC, 64]
        ks_ = k[b].reshape(s, nh, HD)[:, h0:h0 + HPC, :]
        qkT = np.empty((2, HPC * HD, s), ml_dtypes.bfloat16)
        qkT[0] = qs.transpose(1, 2, 0).reshape(HPC * HD, s)
        qkT[1] = ks_.transpose(1, 2, 0).reshape(HPC * HD, s)
        # V' = [V | 1] laid out [128, KS, HPC, 65] fp16
        vh = v[b, :, h0 * HD:(h0 + HPC) * HD].reshape(ks_n, 128, HPC, HD)
        vp = np.ones((128, ks_n, HPC, HD + 1), np.float16)
        vp[:, :, :, :HD] = vh.transpose(1, 0, 2, 3)
        nmT = np.ascontiguousarray((~mask[b]).T).astype(np.float16)
        in_maps.append({"qkT": qkT, "vp": vp, "nmT": nmT})
    return in_maps


def assemble_out(results, s=S, d=D):
    """results[c]["out"]: [NG, 65, QG] bf16 (numerator rows 0:64, Z row 64).
    Normalize, transpose, and merge heads."""
    QG = 1024 if s >= 1024 else s
    NQG = s // QG
    out = np.empty((B, s, d), np.float32)
    for c in range(NCORES):
        b, gc = divmod(c, NCORES // B)
        h0 = HPC * gc
        r = np.asarray(results[c]["out"], dtype=np.float32)
        grps = [(h, qg) for qg in range(NQG) for h in range(HPC)]
        for g, (h, qg) in enumerate(grps):
            num = r[g, :HD, :]                # [64, QG]
            z = r[g, HD, :]                   # [QG]
            blk = (num / z).T                 # [QG, 64]
            col = (h0 + h) * HD
            out[b, qg * QG:(qg + 1) * QG, col:col + HD] = blk
    return out


def kernel(q, k, v, mask):
    nc = _get_nc()
    in_maps = make_in_maps(q, k, v, mask)
    res = run_bass_kernel_spmd(nc, in_maps, list(range(NCORES))).results
    return assemble_out(res)
